# revision 5
# baseline (speedup 1.0000x reference)
"""Trainium2 Bass kernel for nn_Att_mlp_softmax (GNN message passing).

Reference computation:
    e = relu(h @ W1 + b1) @ W2 + b2                       # [N, 1] per-node score
    att = softmax(where(G > 0, e.T broadcast, -9e15))     # row-wise over neighbors
    out = (G.sum(-1))[:, None] * (att @ h)                # degree-rescaled aggregation

Because the pre-softmax score of entry (i, j) depends only on column j, the
masked softmax collapses algebraically:
    att[i, j] = G[i, j] * w[j] / sum_j G[i, j] * w[j],  w = exp(e + ESHIFT)
so with H' = [w * h | w | 1] (N x 130):
    Y = G @ H'
    out = Y[:, 129] * Y[:, :128] / Y[:, 128]
One big [N, N] x [N, 130] matmul replaces the N^2 softmax entirely.

v2 schedule (v1 ran the whole MLP ahead of the main loop in the PE's in-order
queue, so the main loop started ~17 us in and the PE sat HAM-throttled at
1.2 GHz until 32 us):
  * The MLP and the main accumulation are INTERLEAVED per 512-node block:
    z matmul -> relu -> 4 e matmuls -> exp -> that block's 4 main-loop chunks.
    The first main matmul issues as soon as hT tile 0 + G group 0 land (~4 us).
  * PSUM is repacked so both phases coexist: the 8 accumulators live 2-per-bank
    in 4 banks ([128, 264] f32 tiles, acc it at col offset 132*(it%2)), plus
    2 rotating z banks and 1 bank shared by the e columns and warm-up dummies.
  * DMA issues (~740 ns of issuing-engine time each) are spread across the
    sync / scalar / gpsimd queues so no queue serializes the critical path:
    sync gets W1 + even hT tiles + early G groups, scalar gets odd hT tiles +
    W2/b2e (later hT issues drop behind block 0's relu/exp), gpsimd (SWDGE)
    gets b1 + hc quarters + late G groups.
  * G is fully prefetched into SBUF (64 KB/partition -- the whole 8 MB shard
    fits): first groups are 2 chunks so chunk 0 is ready early, later groups
    8 chunks for cheap issues; DMA never idles behind consumption.
  * Warm-up dummy matmuls are tied to early DMA completions (W1, G0) so the
    PE HAM clock-gate flips to 8/8 during the load phase, not 25 us in.
  * The last group runs it-major with staggered stop; each bank's epilogue
    chain + output DMA (alternating sync/scalar) overlaps the loop tail.

Distribution: G is row-sharded across 8 NeuronCores (1024 rows each); h and
the MLP weights are replicated.  Each core's G shard is laid out
[128, JC, ROWS] (contraction-position major) so every DMA line is contiguous.
h is passed twice in bf16: d-major (hT, the MLP moving operand) and
chunk-major (hc, for the H' build).  The output is stored p-major
([128, 8, D]; host inverts).  No collectives.
"""

import numpy as np

N = 8192
D = 128
HID = 64
N_CORES = 8
ROWS = N // N_CORES          # 1024 output rows per core
JC = N // 128                # 64 contraction chunks of 128
NCOL = 130                   # H' columns: [w*h | w | 1]
ESHIFT = -1.0                # exp(e - 1): cancels exactly in the ratio
# G group sizes in chunks: small first groups so chunk 0 lands early, then
# 1 MB transfers for cheap issue + good packets.  Sum must be JC.
G_GROUPS = [2, 2, 4, 8, 8, 8, 8, 8, 8, 8]

_cache = {}


def _install_axon_hooks_shim():
    """Provide antenv.axon_hooks if the image lacks it (trn_boot step 6).

    concourse.bass_utils imports it unconditionally when BASS_TRACE is set;
    without the shim that import crashes instead of degrading.
    """
    import contextlib
    import ctypes
    import sys
    import types

    try:
        import antenv.axon_hooks  # noqa: F401
        return
    except ImportError:
        pass

    so_path = "/opt/axon/libaxon_pjrt.so"

    def _make_hook():
        try:
            lib = ctypes.CDLL(so_path)
        except OSError:
            return None
        if not hasattr(lib, "axon_start_nrt_profile"):
            return None
        lib.axon_start_nrt_profile.argtypes = [
            ctypes.POINTER(ctypes.c_int64),
            ctypes.c_size_t,
        ]
        lib.axon_start_nrt_profile.restype = ctypes.c_int64
        lib.axon_stop_nrt_profile.argtypes = [ctypes.c_char_p]
        lib.axon_stop_nrt_profile.restype = ctypes.c_int64

        @contextlib.contextmanager
        def _hook(output_dir, device_ids):
            import jax

            jax.devices()
            if device_ids:
                ids = (ctypes.c_int64 * len(device_ids))(*device_ids)
                rc = lib.axon_start_nrt_profile(ids, len(device_ids))
            else:
                rc = lib.axon_start_nrt_profile(None, 0)
            if rc != 0:
                raise RuntimeError(f"axon_start_nrt_profile rc={rc}")
            try:
                yield
            finally:
                lib.axon_stop_nrt_profile(str(output_dir).encode())

        return _hook

    mod = types.ModuleType("antenv.axon_hooks")
    _holder = {"hook": _make_hook()}
    mod.set_axon_ntff_profile_hook = lambda h: _holder.__setitem__("hook", h)
    mod.get_axon_ntff_profile_hook = lambda: _holder["hook"]
    sys.modules["antenv.axon_hooks"] = mod
    try:
        import antenv

        antenv.axon_hooks = mod
    except ImportError:
        pass


def build_nc(enable_asserts=False):
    """Build + compile the per-core Bass program (identical on all 8 cores)."""
    from concourse import bacc, mybir, tile

    f32 = mybir.dt.float32
    bf16 = mybir.dt.bfloat16
    f8 = mybir.dt.float8e4
    AF = mybir.ActivationFunctionType

    nc = bacc.Bacc(
        "TRN2",
        target_bir_lowering=False,
        debug=False,
        enable_asserts=enable_asserts,
        num_devices=N_CORES,
    )
    g8 = nc.dram_tensor("g8", [128, JC, ROWS], f8, kind="ExternalInput").ap()
    hT = nc.dram_tensor("hT", [D, N], bf16, kind="ExternalInput").ap()
    hc = nc.dram_tensor("hc", [128, JC, D], bf16, kind="ExternalInput").ap()
    W1 = nc.dram_tensor("W1", [D, HID], bf16, kind="ExternalInput").ap()
    b1 = nc.dram_tensor("b1", [HID, 1], f32, kind="ExternalInput").ap()
    W2 = nc.dram_tensor("W2", [HID, 1], bf16, kind="ExternalInput").ap()
    # b2e = b2 + ESHIFT pre-broadcast to [128, 1] on the host (exp bias)
    b2e = nc.dram_tensor("b2e", [128, 1], f32, kind="ExternalInput").ap()
    out = nc.dram_tensor("out", [128, 8, D], f32, kind="ExternalOutput").ap()

    g_start = [sum(G_GROUPS[:i]) for i in range(len(G_GROUPS))]

    def glookup(jc):
        for gi in range(len(G_GROUPS) - 1, -1, -1):
            if jc >= g_start[gi]:
                return gi, jc - g_start[gi]
        raise AssertionError

    with tile.TileContext(nc) as tc:
        with (
            tc.tile_pool(name="const", bufs=1) as cpool,
            tc.tile_pool(name="big", bufs=1) as bigpool,
            tc.tile_pool(name="gbuf", bufs=len(G_GROUPS)) as gpool,
            tc.tile_pool(name="hpbuf", bufs=JC) as hpool,
            tc.tile_pool(name="outbuf", bufs=1) as opool,
            tc.tile_pool(name="small", bufs=2) as spool,
            tc.tile_pool(name="ps_a6", bufs=6, space="PSUM") as ps_a6,
        ):
            # ---- DMA issue plan (emission order == per-engine queue order).
            # sync:   W1, hT0, G0, hT2, G1, hT4, G2, G3, G4, [out x2 at end]
            # scalar: hT1, W2, b2e, hT3, | relu0.., hT5, .., hT7, [out x2]
            # gpsimd: b1, hc0, hc1, G5, G6, hc2, hc3, G7, G8, G9
            W1_sb = cpool.tile([D, HID], bf16)
            nc.sync.dma_start(W1_sb[:], W1[:])

            NQ = N // 8
            hTq_sb = [bigpool.tile([D, NQ], bf16, name=f"hTq{q}") for q in range(8)]
            nc.sync.dma_start(hTq_sb[0][:], hT[:, 0:NQ])
            nc.scalar.dma_start(hTq_sb[1][:], hT[:, NQ : 2 * NQ])

            gt_sb = []
            for gi, gn in enumerate(G_GROUPS):
                gt_sb.append(gpool.tile([128, gn, ROWS], f8, tag="gt", name=f"gt{gi}"))
            nc.sync.dma_start(gt_sb[0][:], g8[:, g_start[0] : g_start[0] + G_GROUPS[0], :])

            W2_sb = cpool.tile([HID, 1], bf16)
            nc.scalar.dma_start(W2_sb[:], W2[:])
            b2e_sb = cpool.tile([128, 1], f32)
            nc.scalar.dma_start(b2e_sb[:], b2e[:])

            nc.sync.dma_start(hTq_sb[2][:], hT[:, 2 * NQ : 3 * NQ])
            nc.sync.dma_start(gt_sb[1][:], g8[:, g_start[1] : g_start[1] + G_GROUPS[1], :])
            nc.scalar.dma_start(hTq_sb[3][:], hT[:, 3 * NQ : 4 * NQ])
            nc.sync.dma_start(hTq_sb[4][:], hT[:, 4 * NQ : 5 * NQ])
            for gi in (2, 3, 4):
                nc.sync.dma_start(gt_sb[gi][:], g8[:, g_start[gi] : g_start[gi] + G_GROUPS[gi], :])

            b1_sb = cpool.tile([HID, 1], f32)
            nc.gpsimd.dma_start(b1_sb[:], b1[:])
            hc_sb = bigpool.tile([128, JC, D], bf16)
            QW = JC // 4
            for q in (0, 1):
                cl = slice(q * QW, (q + 1) * QW)
                nc.gpsimd.dma_start(hc_sb[:, cl, :], hc[:, cl, :])
            for gi in (5, 6):
                nc.gpsimd.dma_start(gt_sb[gi][:], g8[:, g_start[gi] : g_start[gi] + G_GROUPS[gi], :])
            for q in (2, 3):
                cl = slice(q * QW, (q + 1) * QW)
                nc.gpsimd.dma_start(hc_sb[:, cl, :], hc[:, cl, :])
            for gi in (7, 8, 9):
                nc.gpsimd.dma_start(gt_sb[gi][:], g8[:, g_start[gi] : g_start[gi] + G_GROUPS[gi], :])

            # ---- SBUF working tensors
            a_sb = bigpool.tile([HID, N], bf16)     # relu(h @ W1 + b1)
            w_sb = cpool.tile([128, JC], f32)       # exp(e + ESHIFT)
            wtail = cpool.tile([128, 2, JC], bf16)  # H' tail cols [w, 1]
            warm = cpool.tile([128, 128], bf16)
            nc.vector.memset(warm[:], 0.0)
            nc.vector.memset(wtail[:, 1, :], 1.0)

            # ---- PSUM layout: accumulation groups are BANK-granular (one
            # pending group per 2 KB bank), so only 6 accumulators can stay
            # open while the MLP holds its z bank + e/dummy bank.  Row blocks
            # 6-7 run as a second pure-PE pass over the resident hp/G tiles
            # once the MLP banks free up.
            accs6 = [ps_a6.tile([128, NCOL], f32, tag="acc", name=f"acc{it}")
                     for it in range(6)]
            hp_all = {}

            def build_hp(jc):
                # just-in-time H' chunk build: 2 DVE ops
                hp = hpool.tile([128, NCOL], bf16, tag="hp", name=f"hp{jc}")
                nc.vector.tensor_scalar_mul(
                    hp[:, 0:128], hc_sb[:, jc, :], w_sb[:, jc : jc + 1]
                )
                nc.vector.tensor_copy(hp[:, 128:130], wtail[:, :, jc])
                hp_all[jc] = hp
                return hp

            with tc.tile_pool(name="ps_mlp", bufs=1, space="PSUM") as ps_mlp:
                # e columns + warm-up dummy scratch share one bank
                pe = ps_mlp.tile([128, 192], f32, tag="pe", bufs=1, name="pe")

                def dummy_mm(stat, n=1):
                    # HAM warm-keeping: tiny matmul gated only on `stat`'s DMA
                    for _ in range(n):
                        nc.tensor.matmul(
                            pe[0:64, 64:192], stat, warm[:], start=True,
                            stop=True,
                        )

                dummy_mm(W1_sb[:, 0:64], 3)
                dummy_mm(gt_sb[0][:, 0, 0:64], 2)

                def mlp_block(b):
                    # z -> relu -> 4x e -> exp -> bf16 w tail for the nodes /
                    # chunks of block b.  Single-buffered pz is stall-free:
                    # the e matmuls already serialize on relu.  wtail cast
                    # runs on scalar so the DVE stream stays free for hp
                    # builds.
                    pz = ps_mlp.tile([HID, 512], f32, tag="pz", bufs=1)
                    hsl = slice((b % 2) * 512, (b % 2) * 512 + 512)
                    nc.tensor.matmul(
                        pz[:], W1_sb[:], hTq_sb[b // 2][:, hsl],
                        start=True, stop=True,
                    )
                    sl = slice(b * 512, (b + 1) * 512)
                    nc.scalar.activation(a_sb[:, sl], pz[:], AF.Relu,
                                         bias=b1_sb[:])
                    for c in range(4 * b, 4 * b + 4):
                        nc.tensor.matmul(
                            pe[:, c : c + 1],
                            a_sb[:, c * 128 : (c + 1) * 128],
                            W2_sb[:, 0:1],
                            start=True,
                            stop=True,
                        )
                    ql = slice(4 * b, 4 * b + 4)
                    nc.scalar.activation(w_sb[:, ql], pe[:, ql], AF.Exp,
                                         bias=b2e_sb[:])
                    nc.scalar.activation(wtail[:, 0, ql], w_sb[:, ql], AF.Copy)

                # later hT issues ride the scalar queue behind block 0/1's
                # activations (needed from block 8 on; issued by ~block 3)
                def late_dma(b):
                    if b == 1:
                        nc.scalar.dma_start(hTq_sb[5][:], hT[:, 5 * NQ : 6 * NQ])
                    elif b == 2:
                        nc.scalar.dma_start(hTq_sb[6][:], hT[:, 6 * NQ : 7 * NQ])
                    elif b == 3:
                        nc.scalar.dma_start(hTq_sb[7][:], hT[:, 7 * NQ : 8 * NQ])

                # ---- pass 1: interleaved MLP + accumulation for row blocks 0-5
                for b in range(16):
                    mlp_block(b)
                    late_dma(b)
                    for jc in range(4 * b, 4 * b + 4):
                        hp = build_hp(jc)
                        gi, jci = glookup(jc)
                        for it in range(6):
                            nc.tensor.matmul(
                                accs6[it][:],
                                gt_sb[gi][:, jci, it * 128 : (it + 1) * 128],
                                hp[:],
                                start=(jc == 0),
                                stop=(jc == JC - 1),
                            )

            with tc.tile_pool(name="ps_a2", bufs=2, space="PSUM") as ps_a2:
                # ---- pass 2: row blocks 6-7, it-major so block 6's epilogue
                # + output DMA overlap block 7's sweep
                accs2 = [ps_a2.tile([128, NCOL], f32, tag="acc2",
                                    name=f"acc2_{i}") for i in range(2)]
                for i, it in enumerate((6, 7)):
                    for jc in range(JC):
                        gi, jci = glookup(jc)
                        nc.tensor.matmul(
                            accs2[i][:],
                            gt_sb[gi][:, jci, it * 128 : (it + 1) * 128],
                            hp_all[jc][:],
                            start=(jc == 0),
                            stop=(jc == JC - 1),
                        )

                # epilogue, fully per-bank: each bank's whole chain (tail copy
                # -> recip -> r -> scaled output) runs as soon as ITS
                # accumulator stops; row blocks 0-5 scale + store during pass
                # 2.  Output DMAs alternate sync/scalar queues.
                ot_all = opool.tile([128, 8, D], f32, tag="ot_all", bufs=1)
                for it in range(8):
                    acc = accs6[it] if it < 6 else accs2[it - 6]
                    tl = spool.tile([128, 2], f32, tag="tl", name=f"tl{it}",
                                    bufs=8)
                    nc.vector.tensor_copy(tl[:], acc[:, 128:130])
                    den = spool.tile([128, 1], f32, tag="den", name=f"den{it}",
                                     bufs=8)
                    nc.vector.tensor_scalar_add(den[:], tl[:, 0:1], 1e-30)
                    rc = spool.tile([128, 1], f32, tag="rc", name=f"rc{it}",
                                    bufs=8)
                    nc.vector.reciprocal(rc[:], den[:])
                    r1 = spool.tile([128, 1], f32, tag="r1", name=f"r1{it}",
                                    bufs=8)
                    nc.vector.tensor_mul(r1[:], rc[:], tl[:, 1:2])
                    nc.vector.tensor_scalar_mul(
                        ot_all[:, it, :], acc[:, 0:128], r1[:]
                    )
                    if it in (1, 3, 5):
                        eng = nc.sync if it in (1, 5) else nc.scalar
                        eng.dma_start(
                            out[:, it - 1 : it + 1, :],
                            ot_all[:, it - 1 : it + 1, :],
                        )
                    elif it == 6:
                        nc.sync.dma_start(out[:, 6:7, :], ot_all[:, 6:7, :])
                    elif it == 7:
                        nc.scalar.dma_start(out[:, 7:8, :], ot_all[:, 7:8, :])

    nc.compile()
    return nc


def make_in_maps(graph_info, h, W1, b1, W2, b2):
    """Shard + lay out the full inputs for the 8 cores."""
    import ml_dtypes

    bf16 = ml_dtypes.bfloat16
    f8 = ml_dtypes.float8_e4m3fn

    # G (exact 0/1) as fp8, laid out [core][128 c, JC, ROWS] so the stationary
    # tile for (chunk jc, row block it) is g8[:, jc, it*128:(it+1)*128] and
    # every per-partition DMA line is contiguous
    g = np.asarray(graph_info, np.float32)
    G8 = g.astype(f8).reshape(N_CORES, ROWS, JC, 128).transpose(0, 3, 2, 1)
    h = np.asarray(h, np.float32)
    hTb = np.ascontiguousarray(h.T).astype(bf16)               # [D, N]
    hcb = np.ascontiguousarray(
        h.reshape(JC, 128, D).transpose(1, 0, 2)               # [128, JC, D]
    ).astype(bf16)
    W1b = np.asarray(W1, np.float32).astype(bf16)
    b1r = np.asarray(b1, np.float32).reshape(HID, 1)
    W2r = np.asarray(W2, np.float32).reshape(HID, 1).astype(bf16)
    b2e = np.full((128, 1), float(np.asarray(b2).reshape(())) + ESHIFT,
                  np.float32)
    in_maps = []
    for c in range(N_CORES):
        in_maps.append(
            {
                "g8": np.ascontiguousarray(G8[c]),
                "hT": hTb,
                "hc": hcb,
                "W1": W1b,
                "b1": b1r,
                "W2": W2r,
                "b2e": b2e,
            }
        )
    return in_maps


def kernel(graph_info, h, W1, b1, W2, b2):
    _install_axon_hooks_shim()
    from concourse.bass_utils import run_bass_kernel_spmd

    if "nc" not in _cache:
        _cache["nc"] = build_nc()
    nc = _cache["nc"]

    in_maps = make_in_maps(graph_info, h, W1, b1, W2, b2)
    res = run_bass_kernel_spmd(nc, in_maps, list(range(N_CORES)))
    # out is stored p-major [128, 8, D] per core; invert to row order
    return np.concatenate(
        [
            res.results[c]["out"].transpose(1, 0, 2).reshape(ROWS, D)
            for c in range(N_CORES)
        ],
        axis=0,
    )


# revision 13
# speedup vs baseline: 1.1317x; 1.1317x over previous
"""Trainium2 Bass kernel for nn_Att_mlp_softmax (GNN message passing).

Reference computation:
    e = relu(h @ W1 + b1) @ W2 + b2                       # [N, 1] per-node score
    att = softmax(where(G > 0, e.T broadcast, -9e15))     # row-wise over neighbors
    out = (G.sum(-1))[:, None] * (att @ h)                # degree-rescaled aggregation

Because the pre-softmax score of entry (i, j) depends only on column j, the
masked softmax collapses algebraically:
    att[i, j] = G[i, j] * w[j] / sum_j G[i, j] * w[j],  w = exp(e + ESHIFT)
so with H' = [w * h | w | 1] (N x 130):
    Y = G @ H'
    out = Y[:, 129] * Y[:, :128] / Y[:, 128]
One big [N, N] x [N, 130] matmul replaces the N^2 softmax entirely.

Schedule (the original baseline ran the whole MLP ahead of the main loop in
the PE's in-order queue, so the main loop started ~17 us in and the PE sat
HAM-throttled at 1.2 GHz until 32 us):
  * The MLP and the main accumulation are interleaved per 512-node block with
    a 2-deep software pipeline -- PE queue order z0 z1 e0 [mm_b z_{b+2}
    e_{b+1}] -- so every PE op's producers (relu/exp/cast on scalar, hp on
    DVE) completed >= 1 block earlier and the PE never idles mid-loop.
  * PSUM accumulation groups are bank-granular (one pending group per 2 KB
    bank), so only 6 row-block accumulators coexist with the MLP's z bank +
    e/dummy bank.  Row blocks 6-7 run as a second pure-PE pass over the
    SBUF-resident hp/G tiles once the MLP banks free up; epilogues + output
    DMAs for blocks 0-5 overlap that pass.
  * DMA issues (~740 ns of issuing-engine time each) are spread across the
    sync / scalar / gpsimd queues so no queue serializes the critical path,
    and every transfer's source is its own fully-contiguous DRAM tensor.
  * G is fully prefetched into SBUF (64 KB/partition -- the whole 8 MB shard
    fits): first groups are 2 chunks so chunk 0 is ready early, later groups
    up to 16 chunks (2 MB) for cheap issues; DMA never idles behind
    consumption.
  * Warm-up dummy matmuls tied to the W1 DMA start the PE HAM clock-gate
    warm-up during the load phase.
  * Per-bank epilogue chains (tail copy -> recip -> scale) fire as each
    accumulator stops; output DMAs alternate sync/scalar queues.

Distribution: G is row-sharded across 8 NeuronCores (1024 rows each); h and
the MLP weights are replicated.  Each core's G shard is laid out
[128, JC, ROWS] (contraction-position major) so every DMA line is contiguous.
h is passed twice in bf16: d-major (hT, the MLP moving operand) and
chunk-major (hc, for the H' build).  The output is stored p-major
([128, 8, D]; host inverts).  No collectives.
"""

import numpy as np

N = 8192
D = 128
HID = 64
N_CORES = 8
ROWS = N // N_CORES          # 1024 output rows per core
JC = N // 128                # 64 contraction chunks of 128
NCOL = 130                   # H' columns: [w*h | w | 1]
ESHIFT = -1.0                # exp(e - 1): cancels exactly in the ratio
# G group sizes in chunks: small first groups so chunk 0 lands early, then
# 1-2 MB transfers for cheap issue + good packets.  Sum must be JC.
G_GROUPS = [2, 2, 4, 8, 16, 16, 16]

_cache = {}


def _install_axon_hooks_shim():
    """Provide antenv.axon_hooks if the image lacks it (trn_boot step 6).

    concourse.bass_utils imports it unconditionally when BASS_TRACE is set;
    without the shim that import crashes instead of degrading.
    """
    import contextlib
    import ctypes
    import sys
    import types

    try:
        import antenv.axon_hooks  # noqa: F401
        return
    except ImportError:
        pass

    so_path = "/opt/axon/libaxon_pjrt.so"

    def _make_hook():
        try:
            lib = ctypes.CDLL(so_path)
        except OSError:
            return None
        if not hasattr(lib, "axon_start_nrt_profile"):
            return None
        lib.axon_start_nrt_profile.argtypes = [
            ctypes.POINTER(ctypes.c_int64),
            ctypes.c_size_t,
        ]
        lib.axon_start_nrt_profile.restype = ctypes.c_int64
        lib.axon_stop_nrt_profile.argtypes = [ctypes.c_char_p]
        lib.axon_stop_nrt_profile.restype = ctypes.c_int64

        @contextlib.contextmanager
        def _hook(output_dir, device_ids):
            import jax

            jax.devices()
            if device_ids:
                ids = (ctypes.c_int64 * len(device_ids))(*device_ids)
                rc = lib.axon_start_nrt_profile(ids, len(device_ids))
            else:
                rc = lib.axon_start_nrt_profile(None, 0)
            if rc != 0:
                raise RuntimeError(f"axon_start_nrt_profile rc={rc}")
            try:
                yield
            finally:
                lib.axon_stop_nrt_profile(str(output_dir).encode())

        return _hook

    mod = types.ModuleType("antenv.axon_hooks")
    _holder = {"hook": _make_hook()}
    mod.set_axon_ntff_profile_hook = lambda h: _holder.__setitem__("hook", h)
    mod.get_axon_ntff_profile_hook = lambda: _holder["hook"]
    sys.modules["antenv.axon_hooks"] = mod
    try:
        import antenv

        antenv.axon_hooks = mod
    except ImportError:
        pass


def build_nc(enable_asserts=False):
    """Build + compile the per-core Bass program (identical on all 8 cores)."""
    from concourse import bacc, mybir, tile

    f32 = mybir.dt.float32
    bf16 = mybir.dt.bfloat16
    f8 = mybir.dt.float8e4
    AF = mybir.ActivationFunctionType

    nc = bacc.Bacc(
        "TRN2",
        target_bir_lowering=False,
        debug=False,
        enable_asserts=enable_asserts,
        num_devices=N_CORES,
    )
    # Every DMA source is its own fully-contiguous DRAM tensor (the host
    # pre-slices): with partition stride == line size the whole transfer is
    # one linear HBM stream, vs ~300 GB/s measured with 16-64 KB strides.
    g8 = [
        nc.dram_tensor(f"g8_{gi}", [128, gn, ROWS], f8, kind="ExternalInput").ap()
        for gi, gn in enumerate(G_GROUPS)
    ]
    hts = [
        nc.dram_tensor(f"hts{q}", [D, 512], bf16, kind="ExternalInput").ap()
        for q in range(2)
    ]
    htb = [
        nc.dram_tensor(f"htb{k}", [D, 1024], bf16, kind="ExternalInput").ap()
        for k in range(1, 8)
    ]
    hcq = [
        nc.dram_tensor(f"hcq{q}", [128, JC // 4, D], bf16, kind="ExternalInput").ap()
        for q in range(4)
    ]
    W1 = nc.dram_tensor("W1", [D, HID], bf16, kind="ExternalInput").ap()
    W2 = nc.dram_tensor("W2", [HID, 1], bf16, kind="ExternalInput").ap()
    # cb packs [b2e | b1]: col 0 = b2 + ESHIFT broadcast (exp bias), col 1
    # rows 0-63 = b1 (relu bias) -- one tiny DMA instead of two
    cb = nc.dram_tensor("cb", [128, 2], f32, kind="ExternalInput").ap()
    out = nc.dram_tensor("out", [128, 8, D], f32, kind="ExternalOutput").ap()

    g_start = [sum(G_GROUPS[:i]) for i in range(len(G_GROUPS))]

    def glookup(jc):
        for gi in range(len(G_GROUPS) - 1, -1, -1):
            if jc >= g_start[gi]:
                return gi, jc - g_start[gi]
        raise AssertionError

    with tile.TileContext(nc) as tc:
        with (
            tc.tile_pool(name="const", bufs=1) as cpool,
            tc.tile_pool(name="big", bufs=1) as bigpool,
            tc.tile_pool(name="gbuf", bufs=len(G_GROUPS)) as gpool,
            tc.tile_pool(name="hpbuf", bufs=JC) as hpool,
            tc.tile_pool(name="outbuf", bufs=1) as opool,
            tc.tile_pool(name="small", bufs=2) as spool,
            tc.tile_pool(name="ps_a6", bufs=6, space="PSUM") as ps_a6,
        ):
            # ---- DMA issue plan (emission order == per-engine queue order).
            # sync:   W1, hTs0, G0, hTb1, G1, hTb3, hTb2, G2..G6, [out at end]
            # scalar: hTs1, | relu0.., hTb4..hTb7 (late_dma), [out]
            # gpsimd: cb, W2, hc0..hc3  (SWDGE; tiny consts land by ~2.5 us)
            # hT lives in 2 small 512-node tiles (so z0 starts early) + 7 big
            # 1024-node tiles.  Block b (512 nodes) -> htile(b).
            W1_sb = cpool.tile([D, HID], bf16)
            nc.sync.dma_start(W1_sb[:], W1[:])

            hTs_sb = [bigpool.tile([D, 512], bf16, name=f"hTs{q}") for q in range(2)]
            hTb_sb = [bigpool.tile([D, 1024], bf16, name=f"hTb{q}") for q in range(1, 8)]

            def htile(b):
                if b < 2:
                    return hTs_sb[b], slice(0, 512)
                return hTb_sb[b // 2 - 1], slice((b % 2) * 512, (b % 2) * 512 + 512)

            nc.sync.dma_start(hTs_sb[0][:], hts[0][:])
            nc.scalar.dma_start(hTs_sb[1][:], hts[1][:])

            gt_sb = []
            for gi, gn in enumerate(G_GROUPS):
                gt_sb.append(gpool.tile([128, gn, ROWS], f8, tag="gt", name=f"gt{gi}"))

            def g_issue(gi):
                nc.sync.dma_start(gt_sb[gi][:], g8[gi][:])

            g_issue(0)
            nc.sync.dma_start(hTb_sb[0][:], htb[0][:])
            g_issue(1)
            nc.sync.dma_start(hTb_sb[2][:], htb[2][:])
            nc.sync.dma_start(hTb_sb[1][:], htb[1][:])
            for gi in range(2, len(G_GROUPS)):
                g_issue(gi)

            cb_sb = cpool.tile([128, 2], f32)
            nc.gpsimd.dma_start(cb_sb[:], cb[:])
            b2e_sb = cb_sb[:, 0:1]
            b1_sb = cb_sb[0:HID, 1:2]
            W2_sb = cpool.tile([HID, 1], bf16)
            nc.gpsimd.dma_start(W2_sb[:], W2[:])
            hc_sb = bigpool.tile([128, JC, D], bf16)
            QW = JC // 4
            for q in range(4):
                cl = slice(q * QW, (q + 1) * QW)
                nc.gpsimd.dma_start(hc_sb[:, cl, :], hcq[q][:])

            # ---- SBUF working tensors
            a_sb = bigpool.tile([HID, N], bf16)     # relu(h @ W1 + b1)
            w_sb = cpool.tile([128, JC], f32)       # exp(e + ESHIFT)
            wtail = cpool.tile([128, 2, JC], bf16)  # H' tail cols [w, 1]
            warm = cpool.tile([128, 128], bf16)
            nc.vector.memset(warm[:], 0.0)
            nc.vector.memset(wtail[:, 1, :], 1.0)

            # ---- PSUM layout: accumulation groups are BANK-granular (one
            # pending group per 2 KB bank), so only 6 accumulators can stay
            # open while the MLP holds its z bank + e/dummy bank.  Row blocks
            # 6-7 run as a second pure-PE pass over the resident hp/G tiles
            # once the MLP banks free up.
            accs6 = [ps_a6.tile([128, NCOL], f32, tag="acc", name=f"acc{it}")
                     for it in range(6)]
            hp_all = {}

            def build_hp(jc):
                # just-in-time H' chunk build: 2 DVE ops
                hp = hpool.tile([128, NCOL], bf16, tag="hp", name=f"hp{jc}")
                nc.vector.tensor_scalar_mul(
                    hp[:, 0:128], hc_sb[:, jc, :], w_sb[:, jc : jc + 1]
                )
                nc.vector.tensor_copy(hp[:, 128:130], wtail[:, :, jc])
                hp_all[jc] = hp
                return hp

            with tc.tile_pool(name="ps_mlp", bufs=1, space="PSUM") as ps_mlp:
                # e columns + warm-up dummy scratch share one bank
                pe = ps_mlp.tile([128, 192], f32, tag="pe", bufs=1, name="pe")

                def dummy_mm(stat, n=1):
                    # HAM warm-keeping: tiny matmul gated only on `stat`'s DMA
                    for _ in range(n):
                        nc.tensor.matmul(
                            pe[0:64, 64:192], stat, warm[:], start=True,
                            stop=True,
                        )

                dummy_mm(W1_sb[:, 0:64], 3)

                def mlp_z(b):
                    # z matmul + relu for block b's 512 nodes.  pz is
                    # single-buffered: in the pipelined emission, relu_b
                    # completes during mm_{b-1}, before z_{b+1} reuses the
                    # bank (only z1 pays a one-time ~0.9 us front stall).
                    pz = ps_mlp.tile([HID, 512], f32, tag="pz", bufs=1)
                    t, hsl = htile(b)
                    nc.tensor.matmul(
                        pz[:], W1_sb[:], t[:, hsl], start=True, stop=True
                    )
                    sl = slice(b * 512, (b + 1) * 512)
                    nc.scalar.activation(a_sb[:, sl], pz[:], AF.Relu,
                                         bias=b1_sb)

                def mlp_e(b):
                    # 4x e matmuls -> exp -> bf16 w tail for block b's chunks.
                    # wtail cast runs on scalar so the DVE stream stays free
                    # for hp builds.
                    for c in range(4 * b, 4 * b + 4):
                        nc.tensor.matmul(
                            pe[:, c : c + 1],
                            a_sb[:, c * 128 : (c + 1) * 128],
                            W2_sb[:, 0:1],
                            start=True,
                            stop=True,
                        )
                    ql = slice(4 * b, 4 * b + 4)
                    nc.scalar.activation(w_sb[:, ql], pe[:, ql], AF.Exp,
                                         bias=b2e_sb)
                    nc.scalar.activation(wtail[:, 0, ql], w_sb[:, ql], AF.Copy)

                # later hT issues ride the scalar queue behind the early
                # blocks' activations (needed from block 8 on; issued by ~b=3)
                def late_dma(b):
                    if b <= 3:
                        k = b + 3
                        nc.scalar.dma_start(hTb_sb[k][:], htb[k][:])

                def mm_block(b):
                    for jc in range(4 * b, 4 * b + 4):
                        hp = build_hp(jc)
                        gi, jci = glookup(jc)
                        for it in range(6):
                            nc.tensor.matmul(
                                accs6[it][:],
                                gt_sb[gi][:, jci, it * 128 : (it + 1) * 128],
                                hp[:],
                                start=(jc == 0),
                                stop=(jc == JC - 1),
                            )

                # ---- pass 1: 2-deep software-pipelined MLP + accumulation
                # for row blocks 0-5.  PE queue order z0 z1 e0 [mm_b z_{b+2}
                # e_{b+1}] keeps every PE op's producers >= 1 block ahead.
                mlp_z(0)
                mlp_z(1)
                mlp_e(0)
                for b in range(16):
                    mm_block(b)
                    if b + 2 < 16:
                        mlp_z(b + 2)
                    if b + 1 < 16:
                        mlp_e(b + 1)
                    late_dma(b)

            with tc.tile_pool(name="ps_a2", bufs=2, space="PSUM") as ps_a2:
                # ---- pass 2: row blocks 6-7, it-major so block 6's epilogue
                # + output DMA overlap block 7's sweep
                accs2 = [ps_a2.tile([128, NCOL], f32, tag="acc2",
                                    name=f"acc2_{i}") for i in range(2)]
                for i, it in enumerate((6, 7)):
                    for jc in range(JC):
                        gi, jci = glookup(jc)
                        nc.tensor.matmul(
                            accs2[i][:],
                            gt_sb[gi][:, jci, it * 128 : (it + 1) * 128],
                            hp_all[jc][:],
                            start=(jc == 0),
                            stop=(jc == JC - 1),
                        )

                # epilogue, fully per-bank: each bank's whole chain (tail copy
                # -> recip -> r -> scaled output) runs as soon as ITS
                # accumulator stops; row blocks 0-5 scale + store during pass
                # 2.  Output DMAs alternate sync/scalar queues.
                ot_all = opool.tile([128, 8, D], f32, tag="ot_all", bufs=1)
                for it in range(8):
                    acc = accs6[it] if it < 6 else accs2[it - 6]
                    tl = spool.tile([128, 2], f32, tag="tl", name=f"tl{it}",
                                    bufs=8)
                    nc.vector.tensor_copy(tl[:], acc[:, 128:130])
                    den = spool.tile([128, 1], f32, tag="den", name=f"den{it}",
                                     bufs=8)
                    nc.vector.tensor_scalar_add(den[:], tl[:, 0:1], 1e-30)
                    rc = spool.tile([128, 1], f32, tag="rc", name=f"rc{it}",
                                    bufs=8)
                    nc.vector.reciprocal(rc[:], den[:])
                    r1 = spool.tile([128, 1], f32, tag="r1", name=f"r1{it}",
                                    bufs=8)
                    nc.vector.tensor_mul(r1[:], rc[:], tl[:, 1:2])
                    nc.vector.tensor_scalar_mul(
                        ot_all[:, it, :], acc[:, 0:128], r1[:]
                    )
                    if it in (1, 3, 5):
                        eng = nc.sync if it in (1, 5) else nc.scalar
                        eng.dma_start(
                            out[:, it - 1 : it + 1, :],
                            ot_all[:, it - 1 : it + 1, :],
                        )
                    elif it == 6:
                        nc.sync.dma_start(out[:, 6:7, :], ot_all[:, 6:7, :])
                    elif it == 7:
                        nc.scalar.dma_start(out[:, 7:8, :], ot_all[:, 7:8, :])

    nc.compile()
    return nc


def make_in_maps(graph_info, h, W1, b1, W2, b2):
    """Shard + lay out the full inputs for the 8 cores.

    Every device tensor is a fully-contiguous DRAM block matching exactly one
    DMA transfer (per-group G, per-tile hT, per-quarter hc), so each transfer
    is one linear HBM stream.
    """
    import ml_dtypes

    bf16 = ml_dtypes.bfloat16
    f8 = ml_dtypes.float8_e4m3fn

    # G (exact 0/1) as fp8, laid out [core][128 c, JC, ROWS] so the stationary
    # tile for (chunk jc, row block it) is g8[:, jc, it*128:(it+1)*128]
    g = np.asarray(graph_info, np.float32)
    G8 = g.astype(f8).reshape(N_CORES, ROWS, JC, 128).transpose(0, 3, 2, 1)
    g_start = [sum(G_GROUPS[:i]) for i in range(len(G_GROUPS))]
    h = np.asarray(h, np.float32)
    hTb = np.ascontiguousarray(h.T).astype(bf16)               # [D, N]
    hcb = np.ascontiguousarray(
        h.reshape(JC, 128, D).transpose(1, 0, 2)               # [128, JC, D]
    ).astype(bf16)
    W1b = np.asarray(W1, np.float32).astype(bf16)
    W2r = np.asarray(W2, np.float32).reshape(HID, 1).astype(bf16)
    cb = np.zeros((128, 2), np.float32)
    cb[:, 0] = float(np.asarray(b2).reshape(())) + ESHIFT
    cb[:HID, 1] = np.asarray(b1, np.float32).reshape(HID)
    QW = JC // 4

    common = {"W1": W1b, "W2": W2r, "cb": cb}
    for q in range(2):
        common[f"hts{q}"] = np.ascontiguousarray(hTb[:, q * 512 : (q + 1) * 512])
    for k in range(1, 8):
        common[f"htb{k}"] = np.ascontiguousarray(hTb[:, k * 1024 : (k + 1) * 1024])
    for q in range(4):
        common[f"hcq{q}"] = np.ascontiguousarray(hcb[:, q * QW : (q + 1) * QW, :])

    in_maps = []
    for c in range(N_CORES):
        m = dict(common)
        for gi, gn in enumerate(G_GROUPS):
            m[f"g8_{gi}"] = np.ascontiguousarray(
                G8[c][:, g_start[gi] : g_start[gi] + gn, :]
            )
        in_maps.append(m)
    return in_maps


def kernel(graph_info, h, W1, b1, W2, b2):
    _install_axon_hooks_shim()
    from concourse.bass_utils import run_bass_kernel_spmd

    if "nc" not in _cache:
        _cache["nc"] = build_nc()
    nc = _cache["nc"]

    in_maps = make_in_maps(graph_info, h, W1, b1, W2, b2)
    res = run_bass_kernel_spmd(nc, in_maps, list(range(N_CORES)))
    # out is stored p-major [128, 8, D] per core; invert to row order
    return np.concatenate(
        [
            res.results[c]["out"].transpose(1, 0, 2).reshape(ROWS, D)
            for c in range(N_CORES)
        ],
        axis=0,
    )
